# revision 13
# baseline (speedup 1.0000x reference)
"""Causal multi-head attention (B=4, T=2048, D=1024, H=16, hd=64) on 8 trn2 cores.

Sharding: core c handles batch b=c//2 and head group g=c%2 (8 heads each).
Each core computes qkv projection for its heads, causal attention, and a
partial output projection; the host sums the two partial outputs per batch.

On-device layout is feature-major ("transposed"): x^T, Q^T/K^T [feat, T],
scores S^T [j, i] so softmax normalization folds into the PV matmul via a
ones-block appended to V (the PSUM accumulator rows opposite the O rows
hold the softmax denominator l_i = sum_j exp(S_ij)).
No max-subtraction is needed: S ~ N(0,1) so exp never overflows fp32.
"""

import os
import sys
import subprocess
import tempfile

# Enable the neuron/axon jax platform even if the caller pinned JAX_PLATFORMS=cpu.
# Keeping the caller's platforms first preserves their default backend.
_jp = os.environ.get("JAX_PLATFORMS")
if _jp and "axon" not in _jp and "neuron" not in _jp:
    os.environ["JAX_PLATFORMS"] = _jp + ",axon"

import numpy as np
import ml_dtypes

B, T, D = 4, 2048, 1024
NH, HD = 16, 64
NCORES = 8
HPC = 8          # heads per core
SCALE = HD ** -0.5
BF16 = ml_dtypes.bfloat16

_CACHE = {}


# ---------------------------------------------------------------------------
# Bass program (identical on all cores; data differs per core)
# ---------------------------------------------------------------------------

def _build_nc(repeat=1):
    import concourse.bass as bass
    import concourse.mybir as mybir
    import concourse.tile as tile

    f32 = mybir.dt.float32
    bf16 = mybir.dt.bfloat16
    ADD = mybir.AluOpType.add
    EXP = mybir.ActivationFunctionType.Exp

    nc = bass.Bass()
    xt = nc.dram_tensor("xt", [D, T], bf16, kind="ExternalInput")          # x^T
    wqk = nc.dram_tensor("wqk", [D, 1024], bf16, kind="ExternalInput")     # [d, q|k cols]
    wv = nc.dram_tensor("wv", [D, 512], bf16, kind="ExternalInput")
    wp = nc.dram_tensor("wp", [512, 1024], bf16, kind="ExternalInput")     # proj rows shard
    msk = nc.dram_tensor("msk", [512, 1024], bf16, kind="ExternalInput")   # 4x[128,1024] causal
    bqk = nc.dram_tensor("bqk", [128, 8], f32, kind="ExternalInput")
    bvb = nc.dram_tensor("bvb", [128, 512], f32, kind="ExternalInput")
    bp = nc.dram_tensor("bp", [128, 8], f32, kind="ExternalInput")
    out = nc.dram_tensor("outT", [1024, T], f32, kind="ExternalOutput")    # out^T partial

    with tile.TileContext(nc) as tc:
        with (
            tc.tile_pool(name="persist", bufs=1) as PS,
            tc.tile_pool(name="p1", bufs=3, space="PSUM") as P1,
            tc.tile_pool(name="p2", bufs=2, space="PSUM") as P2,
            tc.tile_pool(name="ptp", bufs=6) as PT,
            tc.tile_pool(name="recp", bufs=2) as REC,
            tc.tile_pool(name="outp", bufs=2) as OST,
        ):
            xt_sb = PS.tile([128, 8, T], bf16, tag="xt")
            wqk_sb = PS.tile([128, 8, 1024], bf16, tag="wqk")
            wv_sb = PS.tile([128, 8, 512], bf16, tag="wv")
            wp_sb = PS.tile([128, 4, 1024], bf16, tag="wp")
            msk_sb = PS.tile([128, 4, 1024], bf16, tag="msk")
            qkt_sb = PS.tile([128, 8, T], bf16, tag="qkt")   # Q^T|K^T per head-pair
            vp_sb = PS.tile([128, 16, 8, 128], bf16, tag="vp")  # V'' per (tc, head)
            yt_sb = PS.tile([128, 4, T], bf16, tag="yt")     # attention out, feat-major
            bqk_sb = PS.tile([128, 8], f32, tag="bqk")
            bvb_sb = PS.tile([128, 512], f32, tag="bvb")
            bp_sb = PS.tile([128, 8], f32, tag="bp")

            for kc in range(8):
                nc.sync.dma_start(xt_sb[:, kc], xt[kc * 128:(kc + 1) * 128, :])
                nc.sync.dma_start(wqk_sb[:, kc], wqk[kc * 128:(kc + 1) * 128, :])
                nc.sync.dma_start(wv_sb[:, kc], wv[kc * 128:(kc + 1) * 128, :])
            nc.sync.dma_start(wp_sb[:], wp[:].rearrange("(kc p) c -> p kc c", p=128))
            nc.sync.dma_start(msk_sb[:], msk[:].rearrange("(k p) i -> p k i", p=128))
            nc.sync.dma_start(bqk_sb[:], bqk[:])
            nc.sync.dma_start(bvb_sb[:], bvb[:])
            nc.sync.dma_start(bp_sb[:], bp[:])

            # V'' slots default to 1.0 (the ones-block that accumulates l_i)
            nc.vector.memset(vp_sb[:], 1.0)

            def qk_part(mc):
                # QKT[:, mc, :] = (x @ wqk_cols)^T for one 128-col block
                for half in (0, 1):
                    ps = P1.tile([128, 1024], f32, tag="ps")
                    for sl in (0, 1):
                        tcol = half * 1024 + sl * 512
                        for kc in range(8):
                            nc.tensor.matmul(
                                ps[:, sl * 512:(sl + 1) * 512],
                                lhsT=wqk_sb[:, kc, mc * 128:(mc + 1) * 128],
                                rhs=xt_sb[:, kc, tcol:tcol + 512],
                                start=(kc == 0), stop=(kc == 7),
                            )
                    nc.vector.tensor_scalar(
                        qkt_sb[:, mc, half * 1024:(half + 1) * 1024],
                        ps[:], bqk_sb[:, mc:mc + 1], None, op0=ADD,
                    )

            def v_part(tc_i):
                pv = P2.tile([128, 512], f32, tag="po")
                for kc in range(8):
                    nc.tensor.matmul(
                        pv[:],
                        lhsT=xt_sb[:, kc, tc_i * 128:(tc_i + 1) * 128],
                        rhs=wv_sb[:, kc, :],
                        start=(kc == 0), stop=(kc == 7),
                    )
                pvr = pv.rearrange("p (hp h2 e) -> p hp h2 e", h2=2, e=64)
                bvr = bvb_sb.rearrange("p (hp h2 e) -> p hp h2 e", h2=2, e=64)
                vpr = vp_sb[:, tc_i].rearrange("p (hp h2) c -> p hp h2 c", h2=2)
                # even heads: V at cols 0:64 (ones at 64:128); odd heads reversed
                nc.vector.tensor_tensor(vpr[:, :, 0, 0:64], pvr[:, :, 0], bvr[:, :, 0], op=ADD)
                nc.vector.tensor_tensor(vpr[:, :, 1, 64:128], pvr[:, :, 1], bvr[:, :, 1], op=ADD)

            def attention(hp):
                for isl in range(4):
                    o0 = P2.tile([128, 512], f32, tag="po")
                    o1 = P2.tile([128, 512], f32, tag="po")
                    oo = (o0, o1)
                    njt = 4 * isl + 4
                    for jt in range(njt):
                        k = jt - 4 * isl
                        off = 128 * k if k > 0 else 0   # causal: i >= 128k needed
                        w = 512 - off
                        ps = P1.tile([128, 1024], f32, tag="ps")
                        for h2 in (0, 1):
                            pr = slice(h2 * 64, h2 * 64 + 64)
                            nc.tensor.matmul(
                                ps[:, h2 * 512 + off:(h2 + 1) * 512],
                                lhsT=qkt_sb[pr, 4 + hp, jt * 128:(jt + 1) * 128],
                                rhs=qkt_sb[pr, hp, isl * 512 + off:(isl + 1) * 512],
                                start=True, stop=True,
                            )
                        pt = PT.tile([128, 1024], bf16, tag="pt")
                        ps2 = ps.rearrange("p (h i) -> p h i", h=2)
                        pt2 = pt.rearrange("p (h i) -> p h i", h=2)
                        nc.scalar.activation(pt2[:, :, off:], ps2[:, :, off:],
                                             EXP, scale=SCALE)
                        if k >= 0:
                            mk2 = msk_sb[:, k].rearrange("p (h i) -> p h i", h=2)
                            nc.vector.tensor_mul(pt2[:, :, off:], pt2[:, :, off:],
                                                 mk2[:, :, off:])
                        for h2 in (0, 1):
                            nc.tensor.matmul(
                                oo[h2][:, off:],
                                lhsT=vp_sb[:, jt, hp * 2 + h2, :],
                                rhs=pt[:, h2 * 512 + off:(h2 + 1) * 512],
                                start=(jt == 0), stop=(jt == njt - 1),
                                skip_group_check=True,
                            )
                    for h2 in (0, 1):
                        # copy PSUM out promptly so the accumulator bank frees
                        # for the next block; normalize from the SBUF copy
                        ocp = REC.tile([128, 512], f32, tag="ocp")
                        nc.vector.tensor_copy(ocp[:], oo[h2][:])
                        rec = REC.tile([128, 512], f32, tag="rec")
                        lo, hi = slice(0, 64), slice(64, 128)
                        osl, lsl = (lo, hi) if h2 == 0 else (hi, lo)
                        nc.vector.reciprocal(rec[lsl, :], ocp[lsl, :])
                        nc.sync.dma_start(rec[osl, :], rec[lsl, :])
                        nc.vector.tensor_mul(
                            yt_sb[osl, hp, isl * 512:(isl + 1) * 512],
                            ocp[osl, :], rec[osl, :],
                        )

            def proj(mc):
                for th in (0, 1):
                    ps = P1.tile([128, 1024], f32, tag="ps")
                    for sl in (0, 1):
                        tcol = th * 1024 + sl * 512
                        for kc in range(4):
                            nc.tensor.matmul(
                                ps[:, sl * 512:(sl + 1) * 512],
                                lhsT=wp_sb[:, kc, mc * 128:(mc + 1) * 128],
                                rhs=yt_sb[:, kc, tcol:tcol + 512],
                                start=(kc == 0), stop=(kc == 3),
                            )
                    os_ = OST.tile([128, 1024], f32, tag="ost")
                    nc.vector.tensor_scalar(os_[:], ps[:], bp_sb[:, mc:mc + 1], None, op0=ADD)
                    nc.sync.dma_start(
                        out[mc * 128:(mc + 1) * 128, th * 1024:(th + 1) * 1024], os_[:]
                    )

            def body():
                qk_part(0)
                qk_part(4)
                for tc_i in range(16):
                    v_part(tc_i)
                for hp in range(4):
                    attention(hp)
                    if hp < 3:
                        qk_part(hp + 1)
                        qk_part(hp + 5)
                for mc in range(8):
                    proj(mc)

            if repeat == 1:
                body()
            else:
                with tc.For_i(0, repeat, 1):
                    body()

    _split_heavy_waits(nc)
    return nc


def _split_heavy_waits(nc, max_waits=1):
    """This walrus build accepts only one sem wait per instruction; move
    excess waits onto inserted same-engine nops."""
    import concourse.mybir as mybir

    for f in nc.m.functions:
        for blk in f.blocks:
            insts = blk.instructions
            i = 0
            while i < len(insts):
                inst = insts[i]
                si = inst.sync_info
                if si is not None and si.on_wait and len(si.on_wait) > max_waits:
                    waits = list(si.on_wait)
                    excess, keep = waits[max_waits:], waits[:max_waits]
                    nops = []
                    for j in range(0, len(excess), max_waits):
                        nop = mybir.InstNoOp(
                            name=nc.get_next_instruction_name(),
                            engine=inst.engine,
                            bass_nofuse=True,
                            sync_info=mybir.SyncInfo(
                                on_wait=excess[j:j + max_waits], on_update=[]),
                        )
                        nc.register_instruction(nop)
                        nops.append(nop)
                    inst.sync_info = mybir.SyncInfo(
                        on_wait=keep, on_update=list(si.on_update))
                    insts[i:i] = nops
                    i += len(nops)
                i += 1


# ---------------------------------------------------------------------------
# Host-side sharding / running
# ---------------------------------------------------------------------------

def _causal_masks():
    j = np.arange(128)[:, None]
    i = np.arange(512)[None, :]
    ms = []
    for k in range(4):
        m = (i >= 128 * k + j).astype(np.float32)
        ms.append(np.concatenate([m, m], axis=1))  # both head halves
    return np.concatenate(ms, axis=0).astype(BF16)  # [512, 1024]


def _core_inputs(x, w_qkv, b_qkv, w_proj, b_proj):
    msk = _causal_masks()
    maps = []
    for c in range(NCORES):
        b, g = divmod(c, 2)
        cq = slice(g * 512, (g + 1) * 512)
        ck = slice(1024 + g * 512, 1024 + (g + 1) * 512)
        cv = slice(2048 + g * 512, 2048 + (g + 1) * 512)
        wqk = np.concatenate([w_qkv[:, cq], w_qkv[:, ck]], axis=1)
        bqkc = np.concatenate([b_qkv[cq], b_qkv[ck]])
        maps.append({
            "xt": np.ascontiguousarray(x[b].T).astype(BF16),
            "wqk": np.ascontiguousarray(wqk).astype(BF16),
            "wv": np.ascontiguousarray(w_qkv[:, cv]).astype(BF16),
            "wp": np.ascontiguousarray(w_proj[g * 512:(g + 1) * 512]).astype(BF16),
            "msk": msk,
            "bqk": np.ascontiguousarray(bqkc.reshape(8, 128).T).astype(np.float32),
            "bvb": np.broadcast_to(b_qkv[cv], (128, 512)).astype(np.float32),
            "bp": np.ascontiguousarray(b_proj.reshape(8, 128).T).astype(np.float32),
        })
    return maps


def _compile(repeat=1, donate=True):
    key = (repeat, donate)
    if key in _CACHE:
        return _CACHE[key]

    import jax
    from jax.sharding import Mesh, PartitionSpec
    from jax.experimental.shard_map import shard_map
    import concourse.mybir as mybir
    from concourse import bass2jax

    try:
        devices = jax.devices("axon")
    except Exception:
        devices = [d for d in jax.devices() if d.platform != "cpu"]
    if len(devices) < NCORES:
        raise RuntimeError(f"need {NCORES} neuron devices, have {devices}")
    devices = devices[:NCORES]

    bass2jax.install_neuronx_cc_hook()
    nckey = ("nc", repeat)
    if nckey not in _CACHE:
        _CACHE[nckey] = _build_nc(repeat=repeat)
    nc = _CACHE[nckey]
    partition_name = nc.partition_id_tensor.name if nc.partition_id_tensor else None

    in_names, out_names, out_avals, zero_outs = [], [], [], []
    for alloc in nc.m.functions[0].allocations:
        if not isinstance(alloc, mybir.MemoryLocationSet):
            continue
        name = alloc.memorylocations[0].name
        if alloc.kind == "ExternalInput":
            if name != partition_name:
                in_names.append(name)
        elif alloc.kind == "ExternalOutput":
            out_names.append(name)
            shape = tuple(alloc.tensor_shape)
            dtype = mybir.dt.np(alloc.dtype)
            out_avals.append(jax.core.ShapedArray(shape, dtype))
            zero_outs.append(np.zeros(shape, dtype))
    n_params = len(in_names)
    all_names = in_names + out_names
    if partition_name is not None:
        all_names = all_names + [partition_name]

    def _body(*args):
        operands = list(args)
        if partition_name is not None:
            operands.append(bass2jax.partition_id_tensor())
        outs = bass2jax._bass_exec_p.bind(
            *operands,
            out_avals=tuple(out_avals),
            in_names=tuple(all_names),
            out_names=tuple(out_names),
            lowering_input_output_aliases=(),
            sim_require_finite=True,
            sim_require_nnan=True,
            nc=nc,
        )
        return tuple(outs)

    mesh = Mesh(np.asarray(devices), ("core",))
    n_out = len(out_names)
    sharded = jax.jit(
        shard_map(
            _body, mesh=mesh,
            in_specs=(PartitionSpec("core"),) * (n_params + n_out),
            out_specs=(PartitionSpec("core"),) * n_out,
            check_rep=False,
        ),
        donate_argnums=tuple(range(n_params, n_params + n_out)) if donate else (),
        keep_unused=True,
    )
    _CACHE[key] = (sharded, in_names, out_names, out_avals, zero_outs, mesh)
    return _CACHE[key]


def _timed_device(core_maps, iters=10, repeat=1):
    """Wall-time repeated executions with device-resident operands (no
    donation) to approximate pure device execution time."""
    import jax
    import time
    from jax.sharding import PartitionSpec, NamedSharding

    fn, in_names, out_names, out_avals, zero_outs, mesh = _compile(
        repeat=repeat, donate=False)
    sh = NamedSharding(mesh, PartitionSpec("core"))
    dev_in = [
        jax.device_put(
            np.concatenate([np.asarray(core_maps[c][n]) for c in range(NCORES)], axis=0), sh)
        for n in in_names
    ]
    dev_zero = [
        jax.device_put(np.zeros((NCORES * z.shape[0], *z.shape[1:]), z.dtype), sh)
        for z in zero_outs
    ]
    jax.block_until_ready(fn(*dev_in, *dev_zero))  # warm
    times = []
    for _ in range(iters):
        t0 = time.perf_counter()
        jax.block_until_ready(fn(*dev_in, *dev_zero))
        times.append(time.perf_counter() - t0)
    return times


def _run_device(core_maps):
    sharded, in_names, out_names, out_avals, zero_outs, _mesh = _compile()
    concat_in = [
        np.concatenate([np.asarray(core_maps[c][n]) for c in range(NCORES)], axis=0)
        for n in in_names
    ]
    concat_zero = [
        np.zeros((NCORES * z.shape[0], *z.shape[1:]), z.dtype) for z in zero_outs
    ]
    outs = sharded(*concat_in, *concat_zero)
    res = []
    for c in range(NCORES):
        res.append({
            n: np.asarray(outs[i]).reshape(NCORES, *out_avals[i].shape)[c]
            for i, n in enumerate(out_names)
        })
    return res


def _kernel_local(x, w_qkv, b_qkv, w_proj, b_proj):
    maps = _core_inputs(x, w_qkv, b_qkv, w_proj, b_proj)
    res = _run_device(maps)
    out = np.empty((B, T, D), np.float32)
    for b in range(B):
        out[b] = (res[2 * b]["outT"] + res[2 * b + 1]["outT"]).T
    return out


def _kernel_subprocess(x, w_qkv, b_qkv, w_proj, b_proj):
    env = dict(os.environ)
    env.pop("JAX_PLATFORMS", None)
    env["KERNEL_NO_SUBPROC"] = "1"
    with tempfile.TemporaryDirectory() as td:
        np.savez(os.path.join(td, "in.npz"), x=x, w_qkv=w_qkv, b_qkv=b_qkv,
                 w_proj=w_proj, b_proj=b_proj)
        subprocess.run(
            [sys.executable, os.path.abspath(__file__), "--subproc", td],
            env=env, check=True,
        )
        return np.load(os.path.join(td, "out.npy"))


def kernel(x, w_qkv, b_qkv, w_proj, b_proj):
    x = np.asarray(x, np.float32)
    w_qkv = np.asarray(w_qkv, np.float32)
    b_qkv = np.asarray(b_qkv, np.float32)
    w_proj = np.asarray(w_proj, np.float32)
    b_proj = np.asarray(b_proj, np.float32)
    if os.environ.get("KERNEL_NO_SUBPROC"):
        return _kernel_local(x, w_qkv, b_qkv, w_proj, b_proj)
    try:
        return _kernel_local(x, w_qkv, b_qkv, w_proj, b_proj)
    except Exception as e:
        print(f"kernel: in-process device run failed ({type(e).__name__}: {e}); "
              "retrying in a clean subprocess", file=sys.stderr)
        return _kernel_subprocess(x, w_qkv, b_qkv, w_proj, b_proj)


if __name__ == "__main__":
    if len(sys.argv) == 3 and sys.argv[1] == "--subproc":
        td = sys.argv[2]
        data = np.load(os.path.join(td, "in.npz"))
        out = _kernel_local(**{k: data[k] for k in data.files})
        np.save(os.path.join(td, "out.npy"), out)


# revision 14
# speedup vs baseline: 167.9132x; 167.9132x over previous
"""Causal multi-head attention (B=4, T=2048, D=1024, H=16, hd=64) on 8 trn2 cores.

Sharding: core c handles batch b=c//2 and head group g=c%2 (8 heads each).
Each core computes qkv projection for its heads, causal attention, and a
partial output projection; the host sums the two partial outputs per batch.

On-device layout is feature-major ("transposed"): x^T, Q^T/K^T [feat, T],
scores S^T [j, i] so softmax normalization folds into the PV matmul via a
ones-block appended to V (the PSUM accumulator rows opposite the O rows
hold the softmax denominator l_i = sum_j exp(S_ij)).
No max-subtraction is needed: S ~ N(0,1) so exp never overflows fp32.
"""

import os
import sys
import subprocess
import tempfile

# Enable the neuron/axon jax platform even if the caller pinned JAX_PLATFORMS=cpu.
# Keeping the caller's platforms first preserves their default backend.
_jp = os.environ.get("JAX_PLATFORMS")
if _jp and "axon" not in _jp and "neuron" not in _jp:
    os.environ["JAX_PLATFORMS"] = _jp + ",axon"

import numpy as np
import ml_dtypes

B, T, D = 4, 2048, 1024
NH, HD = 16, 64
NCORES = 8
HPC = 8          # heads per core
SCALE = HD ** -0.5
BF16 = ml_dtypes.bfloat16

_CACHE = {}


# ---------------------------------------------------------------------------
# Bass program (identical on all cores; data differs per core)
# ---------------------------------------------------------------------------

def _build_nc(repeat=1):
    import concourse.bass as bass
    import concourse.mybir as mybir
    import concourse.tile as tile

    f32 = mybir.dt.float32
    bf16 = mybir.dt.bfloat16
    ADD = mybir.AluOpType.add
    EXP = mybir.ActivationFunctionType.Exp

    nc = bass.Bass()
    xt = nc.dram_tensor("xt", [D, T], bf16, kind="ExternalInput")          # x^T
    wqk = nc.dram_tensor("wqk", [D, 1024], bf16, kind="ExternalInput")     # [d, q|k cols]
    wv = nc.dram_tensor("wv", [D, 512], bf16, kind="ExternalInput")
    wp = nc.dram_tensor("wp", [512, 1024], bf16, kind="ExternalInput")     # proj rows shard
    msk = nc.dram_tensor("msk", [512, 1024], bf16, kind="ExternalInput")   # 4x[128,1024] causal
    bqk = nc.dram_tensor("bqk", [128, 8], f32, kind="ExternalInput")
    bvb = nc.dram_tensor("bvb", [128, 512], f32, kind="ExternalInput")
    bp = nc.dram_tensor("bp", [128, 8], f32, kind="ExternalInput")
    out = nc.dram_tensor("outT", [1024, T], f32, kind="ExternalOutput")    # out^T partial

    with tile.TileContext(nc) as tc:
        with (
            tc.tile_pool(name="persist", bufs=1) as PS,
            tc.tile_pool(name="p1", bufs=3, space="PSUM") as P1,
            tc.tile_pool(name="p2", bufs=2, space="PSUM") as P2,
            tc.tile_pool(name="ptp", bufs=6) as PT,
            tc.tile_pool(name="recp", bufs=2) as REC,
            tc.tile_pool(name="outp", bufs=2) as OST,
        ):
            xt_sb = PS.tile([128, 8, T], bf16, tag="xt")
            wqk_sb = PS.tile([128, 8, 1024], bf16, tag="wqk")
            wv_sb = PS.tile([128, 8, 512], bf16, tag="wv")
            wp_sb = PS.tile([128, 4, 1024], bf16, tag="wp")
            msk_sb = PS.tile([128, 4, 1024], bf16, tag="msk")
            qkt_sb = PS.tile([128, 8, T], bf16, tag="qkt")   # Q^T|K^T per head-pair
            vp_sb = PS.tile([128, 16, 8, 128], bf16, tag="vp")  # V'' per (tc, head)
            yt_sb = PS.tile([128, 4, T], bf16, tag="yt")     # attention out, feat-major
            bqk_sb = PS.tile([128, 8], f32, tag="bqk")
            bvb_sb = PS.tile([128, 512], f32, tag="bvb")
            bp_sb = PS.tile([128, 8], f32, tag="bp")

            for kc in range(8):
                nc.sync.dma_start(xt_sb[:, kc], xt[kc * 128:(kc + 1) * 128, :])
                nc.sync.dma_start(wqk_sb[:, kc], wqk[kc * 128:(kc + 1) * 128, :])
                nc.sync.dma_start(wv_sb[:, kc], wv[kc * 128:(kc + 1) * 128, :])
            nc.sync.dma_start(wp_sb[:], wp[:].rearrange("(kc p) c -> p kc c", p=128))
            nc.sync.dma_start(msk_sb[:], msk[:].rearrange("(k p) i -> p k i", p=128))
            nc.sync.dma_start(bqk_sb[:], bqk[:])
            nc.sync.dma_start(bvb_sb[:], bvb[:])
            nc.sync.dma_start(bp_sb[:], bp[:])

            # V'' slots default to 1.0 (the ones-block that accumulates l_i)
            nc.vector.memset(vp_sb[:], 1.0)

            def qk_part(mc):
                # QKT[:, mc, :] = (x @ wqk_cols)^T for one 128-col block
                for half in (0, 1):
                    ps = P1.tile([128, 1024], f32, tag="ps")
                    for sl in (0, 1):
                        tcol = half * 1024 + sl * 512
                        for kc in range(8):
                            nc.tensor.matmul(
                                ps[:, sl * 512:(sl + 1) * 512],
                                lhsT=wqk_sb[:, kc, mc * 128:(mc + 1) * 128],
                                rhs=xt_sb[:, kc, tcol:tcol + 512],
                                start=(kc == 0), stop=(kc == 7),
                            )
                    nc.vector.tensor_scalar(
                        qkt_sb[:, mc, half * 1024:(half + 1) * 1024],
                        ps[:], bqk_sb[:, mc:mc + 1], None, op0=ADD,
                    )

            def v_part(tc_i):
                pv = P2.tile([128, 512], f32, tag="po")
                for kc in range(8):
                    nc.tensor.matmul(
                        pv[:],
                        lhsT=xt_sb[:, kc, tc_i * 128:(tc_i + 1) * 128],
                        rhs=wv_sb[:, kc, :],
                        start=(kc == 0), stop=(kc == 7),
                    )
                pvr = pv.rearrange("p (hp h2 e) -> p hp h2 e", h2=2, e=64)
                bvr = bvb_sb.rearrange("p (hp h2 e) -> p hp h2 e", h2=2, e=64)
                vpr = vp_sb[:, tc_i].rearrange("p (hp h2) c -> p hp h2 c", h2=2)
                # even heads: V at cols 0:64 (ones at 64:128); odd heads reversed
                nc.vector.tensor_tensor(vpr[:, :, 0, 0:64], pvr[:, :, 0], bvr[:, :, 0], op=ADD)
                nc.vector.tensor_tensor(vpr[:, :, 1, 64:128], pvr[:, :, 1], bvr[:, :, 1], op=ADD)

            def attention(hp):
                for isl in range(4):
                    o0 = P2.tile([128, 512], f32, tag="po")
                    o1 = P2.tile([128, 512], f32, tag="po")
                    oo = (o0, o1)
                    njt = 4 * isl + 4
                    for jt in range(njt):
                        k = jt - 4 * isl
                        off = 128 * k if k > 0 else 0   # causal: i >= 128k needed
                        w = 512 - off
                        ps = P1.tile([128, 1024], f32, tag="ps")
                        for h2 in (0, 1):
                            pr = slice(h2 * 64, h2 * 64 + 64)
                            nc.tensor.matmul(
                                ps[:, h2 * 512 + off:(h2 + 1) * 512],
                                lhsT=qkt_sb[pr, 4 + hp, jt * 128:(jt + 1) * 128],
                                rhs=qkt_sb[pr, hp, isl * 512 + off:(isl + 1) * 512],
                                start=True, stop=True,
                            )
                        pt = PT.tile([128, 1024], bf16, tag="pt")
                        ps2 = ps.rearrange("p (h i) -> p h i", h=2)
                        pt2 = pt.rearrange("p (h i) -> p h i", h=2)
                        nc.scalar.activation(pt2[:, :, off:], ps2[:, :, off:],
                                             EXP, scale=SCALE)
                        if k >= 0:
                            # only the 128-col diagonal band needs masking;
                            # columns beyond it are fully allowed
                            mk2 = msk_sb[:, k].rearrange("p (h i) -> p h i", h=2)
                            nc.vector.tensor_mul(
                                pt2[:, :, off:off + 128], pt2[:, :, off:off + 128],
                                mk2[:, :, off:off + 128])
                        for h2 in (0, 1):
                            nc.tensor.matmul(
                                oo[h2][:, off:],
                                lhsT=vp_sb[:, jt, hp * 2 + h2, :],
                                rhs=pt[:, h2 * 512 + off:(h2 + 1) * 512],
                                start=(jt == 0), stop=(jt == njt - 1),
                                skip_group_check=True,
                            )
                    for h2 in (0, 1):
                        # copy PSUM out promptly so the accumulator bank frees
                        # for the next block; normalize from the SBUF copy
                        ocp = REC.tile([128, 512], f32, tag="ocp")
                        nc.vector.tensor_copy(ocp[:], oo[h2][:])
                        rec = REC.tile([128, 512], f32, tag="rec")
                        lo, hi = slice(0, 64), slice(64, 128)
                        osl, lsl = (lo, hi) if h2 == 0 else (hi, lo)
                        nc.vector.reciprocal(rec[lsl, :], ocp[lsl, :])
                        nc.sync.dma_start(rec[osl, :], rec[lsl, :])
                        nc.vector.tensor_mul(
                            yt_sb[osl, hp, isl * 512:(isl + 1) * 512],
                            ocp[osl, :], rec[osl, :],
                        )

            def proj(mc):
                for th in (0, 1):
                    ps = P1.tile([128, 1024], f32, tag="ps")
                    for sl in (0, 1):
                        tcol = th * 1024 + sl * 512
                        for kc in range(4):
                            nc.tensor.matmul(
                                ps[:, sl * 512:(sl + 1) * 512],
                                lhsT=wp_sb[:, kc, mc * 128:(mc + 1) * 128],
                                rhs=yt_sb[:, kc, tcol:tcol + 512],
                                start=(kc == 0), stop=(kc == 3),
                            )
                    os_ = OST.tile([128, 1024], f32, tag="ost")
                    nc.vector.tensor_scalar(os_[:], ps[:], bp_sb[:, mc:mc + 1], None, op0=ADD)
                    nc.sync.dma_start(
                        out[mc * 128:(mc + 1) * 128, th * 1024:(th + 1) * 1024], os_[:]
                    )

            def body():
                qk_part(0)
                qk_part(4)
                for tc_i in range(16):
                    v_part(tc_i)
                for hp in range(4):
                    attention(hp)
                    if hp < 3:
                        qk_part(hp + 1)
                        qk_part(hp + 5)
                for mc in range(8):
                    proj(mc)

            if repeat == 1:
                body()
            else:
                with tc.For_i(0, repeat, 1):
                    body()

    _split_heavy_waits(nc)
    return nc


def _split_heavy_waits(nc, max_waits=1):
    """This walrus build accepts only one sem wait per instruction; move
    excess waits onto inserted same-engine nops."""
    import concourse.mybir as mybir

    for f in nc.m.functions:
        for blk in f.blocks:
            insts = blk.instructions
            i = 0
            while i < len(insts):
                inst = insts[i]
                si = inst.sync_info
                if si is not None and si.on_wait and len(si.on_wait) > max_waits:
                    waits = list(si.on_wait)
                    excess, keep = waits[max_waits:], waits[:max_waits]
                    nops = []
                    for j in range(0, len(excess), max_waits):
                        nop = mybir.InstNoOp(
                            name=nc.get_next_instruction_name(),
                            engine=inst.engine,
                            bass_nofuse=True,
                            sync_info=mybir.SyncInfo(
                                on_wait=excess[j:j + max_waits], on_update=[]),
                        )
                        nc.register_instruction(nop)
                        nops.append(nop)
                    inst.sync_info = mybir.SyncInfo(
                        on_wait=keep, on_update=list(si.on_update))
                    insts[i:i] = nops
                    i += len(nops)
                i += 1


# ---------------------------------------------------------------------------
# Host-side sharding / running
# ---------------------------------------------------------------------------

def _causal_masks():
    j = np.arange(128)[:, None]
    i = np.arange(512)[None, :]
    ms = []
    for k in range(4):
        m = (i >= 128 * k + j).astype(np.float32)
        ms.append(np.concatenate([m, m], axis=1))  # both head halves
    return np.concatenate(ms, axis=0).astype(BF16)  # [512, 1024]


def _core_inputs(x, w_qkv, b_qkv, w_proj, b_proj):
    msk = _causal_masks()
    maps = []
    for c in range(NCORES):
        b, g = divmod(c, 2)
        cq = slice(g * 512, (g + 1) * 512)
        ck = slice(1024 + g * 512, 1024 + (g + 1) * 512)
        cv = slice(2048 + g * 512, 2048 + (g + 1) * 512)
        wqk = np.concatenate([w_qkv[:, cq], w_qkv[:, ck]], axis=1)
        bqkc = np.concatenate([b_qkv[cq], b_qkv[ck]])
        maps.append({
            "xt": np.ascontiguousarray(x[b].T).astype(BF16),
            "wqk": np.ascontiguousarray(wqk).astype(BF16),
            "wv": np.ascontiguousarray(w_qkv[:, cv]).astype(BF16),
            "wp": np.ascontiguousarray(w_proj[g * 512:(g + 1) * 512]).astype(BF16),
            "msk": msk,
            "bqk": np.ascontiguousarray(bqkc.reshape(8, 128).T).astype(np.float32),
            "bvb": np.broadcast_to(b_qkv[cv], (128, 512)).astype(np.float32),
            "bp": np.ascontiguousarray(b_proj.reshape(8, 128).T).astype(np.float32),
        })
    return maps


def _compile(repeat=1, donate=True):
    key = (repeat, donate)
    if key in _CACHE:
        return _CACHE[key]

    import jax
    from jax.sharding import Mesh, PartitionSpec
    from jax.experimental.shard_map import shard_map
    import concourse.mybir as mybir
    from concourse import bass2jax

    try:
        devices = jax.devices("axon")
    except Exception:
        devices = [d for d in jax.devices() if d.platform != "cpu"]
    if len(devices) < NCORES:
        raise RuntimeError(f"need {NCORES} neuron devices, have {devices}")
    devices = devices[:NCORES]

    bass2jax.install_neuronx_cc_hook()
    nckey = ("nc", repeat)
    if nckey not in _CACHE:
        _CACHE[nckey] = _build_nc(repeat=repeat)
    nc = _CACHE[nckey]
    partition_name = nc.partition_id_tensor.name if nc.partition_id_tensor else None

    in_names, out_names, out_avals, zero_outs = [], [], [], []
    for alloc in nc.m.functions[0].allocations:
        if not isinstance(alloc, mybir.MemoryLocationSet):
            continue
        name = alloc.memorylocations[0].name
        if alloc.kind == "ExternalInput":
            if name != partition_name:
                in_names.append(name)
        elif alloc.kind == "ExternalOutput":
            out_names.append(name)
            shape = tuple(alloc.tensor_shape)
            dtype = mybir.dt.np(alloc.dtype)
            out_avals.append(jax.core.ShapedArray(shape, dtype))
            zero_outs.append(np.zeros(shape, dtype))
    n_params = len(in_names)
    all_names = in_names + out_names
    if partition_name is not None:
        all_names = all_names + [partition_name]

    def _body(*args):
        operands = list(args)
        if partition_name is not None:
            operands.append(bass2jax.partition_id_tensor())
        outs = bass2jax._bass_exec_p.bind(
            *operands,
            out_avals=tuple(out_avals),
            in_names=tuple(all_names),
            out_names=tuple(out_names),
            lowering_input_output_aliases=(),
            sim_require_finite=True,
            sim_require_nnan=True,
            nc=nc,
        )
        return tuple(outs)

    mesh = Mesh(np.asarray(devices), ("core",))
    n_out = len(out_names)
    sharded = jax.jit(
        shard_map(
            _body, mesh=mesh,
            in_specs=(PartitionSpec("core"),) * (n_params + n_out),
            out_specs=(PartitionSpec("core"),) * n_out,
            check_rep=False,
        ),
        donate_argnums=tuple(range(n_params, n_params + n_out)) if donate else (),
        keep_unused=True,
    )
    _CACHE[key] = (sharded, in_names, out_names, out_avals, zero_outs, mesh)
    return _CACHE[key]


def _timed_device(core_maps, iters=10, repeat=1):
    """Wall-time repeated executions with device-resident operands (no
    donation) to approximate pure device execution time."""
    import jax
    import time
    from jax.sharding import PartitionSpec, NamedSharding

    fn, in_names, out_names, out_avals, zero_outs, mesh = _compile(
        repeat=repeat, donate=False)
    sh = NamedSharding(mesh, PartitionSpec("core"))
    dev_in = [
        jax.device_put(
            np.concatenate([np.asarray(core_maps[c][n]) for c in range(NCORES)], axis=0), sh)
        for n in in_names
    ]
    dev_zero = [
        jax.device_put(np.zeros((NCORES * z.shape[0], *z.shape[1:]), z.dtype), sh)
        for z in zero_outs
    ]
    jax.block_until_ready(fn(*dev_in, *dev_zero))  # warm
    times = []
    for _ in range(iters):
        t0 = time.perf_counter()
        jax.block_until_ready(fn(*dev_in, *dev_zero))
        times.append(time.perf_counter() - t0)
    return times


def _run_device(core_maps):
    sharded, in_names, out_names, out_avals, zero_outs, _mesh = _compile()
    concat_in = [
        np.concatenate([np.asarray(core_maps[c][n]) for c in range(NCORES)], axis=0)
        for n in in_names
    ]
    concat_zero = [
        np.zeros((NCORES * z.shape[0], *z.shape[1:]), z.dtype) for z in zero_outs
    ]
    outs = sharded(*concat_in, *concat_zero)
    res = []
    for c in range(NCORES):
        res.append({
            n: np.asarray(outs[i]).reshape(NCORES, *out_avals[i].shape)[c]
            for i, n in enumerate(out_names)
        })
    return res


def _kernel_local(x, w_qkv, b_qkv, w_proj, b_proj):
    maps = _core_inputs(x, w_qkv, b_qkv, w_proj, b_proj)
    res = _run_device(maps)
    out = np.empty((B, T, D), np.float32)
    for b in range(B):
        out[b] = (res[2 * b]["outT"] + res[2 * b + 1]["outT"]).T
    return out


def _kernel_subprocess(x, w_qkv, b_qkv, w_proj, b_proj):
    env = dict(os.environ)
    env.pop("JAX_PLATFORMS", None)
    env["KERNEL_NO_SUBPROC"] = "1"
    with tempfile.TemporaryDirectory() as td:
        np.savez(os.path.join(td, "in.npz"), x=x, w_qkv=w_qkv, b_qkv=b_qkv,
                 w_proj=w_proj, b_proj=b_proj)
        subprocess.run(
            [sys.executable, os.path.abspath(__file__), "--subproc", td],
            env=env, check=True,
        )
        return np.load(os.path.join(td, "out.npy"))


def kernel(x, w_qkv, b_qkv, w_proj, b_proj):
    x = np.asarray(x, np.float32)
    w_qkv = np.asarray(w_qkv, np.float32)
    b_qkv = np.asarray(b_qkv, np.float32)
    w_proj = np.asarray(w_proj, np.float32)
    b_proj = np.asarray(b_proj, np.float32)
    if os.environ.get("KERNEL_NO_SUBPROC"):
        return _kernel_local(x, w_qkv, b_qkv, w_proj, b_proj)
    try:
        return _kernel_local(x, w_qkv, b_qkv, w_proj, b_proj)
    except Exception as e:
        print(f"kernel: in-process device run failed ({type(e).__name__}: {e}); "
              "retrying in a clean subprocess", file=sys.stderr)
        return _kernel_subprocess(x, w_qkv, b_qkv, w_proj, b_proj)


if __name__ == "__main__":
    if len(sys.argv) == 3 and sys.argv[1] == "--subproc":
        td = sys.argv[2]
        data = np.load(os.path.join(td, "in.npz"))
        out = _kernel_local(**{k: data[k] for k in data.files})
        np.save(os.path.join(td, "out.npy"), out)
